# revision 2
# baseline (speedup 1.0000x reference)
"""Eval-mode ClassConditionalBatchNorm2d on 8 Trainium2 NeuronCores.

Math: for each sample b with label l:
    use_class = (alpha > 0) & (class_counts[l] >= 100)
    mean/var  = blend of (global, class[l]) stats if use_class else global
    out       = (x - mean) / sqrt(var + eps) * weight + bias

This folds to a per-(sample, channel) affine:  out = x * scale + shift with
    scale[b,c] = weight[c] / sqrt(var[b,c] + eps)
    shift[b,c] = bias[c] - mean[b,c] * scale[b,c]

The [B=64, C=256] scale/shift tables are tiny and computed on host; the
device kernel streams x through SBUF applying one fused DVE tensor_scalar
(mult+add, per-partition scalars) per channel-half — purely memory bound.

Traffic optimization: the accuracy budget (rel err < 2e-2 vs the f32
reference) is far looser than f32, so x crosses HBM as float16 (host casts
x -> f16, device reads f16, computes via DVE, writes f16 out, host upcasts).
f16 rounding of x and out each contribute <= 2^-11 relative error (~5e-4
scale-relative total) while halving HBM bytes: 12.85 MB in + 12.85 MB out
per core instead of 25.7 + 25.7.

Sharding: pure data parallel over batch. Each of the 8 cores gets 8 samples
(x shard [8, 256, 56*56]) plus its own [128, 32] scale/shift table arranged
so that column 4*b + 2*h + {0,1} holds (scale, shift) for sample b, channel
half h, with channels on partitions. Tiles cover one whole sample
([128 partitions, 2 halves, 3136 spatial]) so each load/store is a single
large DMA that fans across the SDMA ports; input + output pools pipeline
load/compute/store.
"""

import numpy as np
from contextlib import ExitStack

B, C, H, W = 64, 256, 56, 56
HW = H * W
N_CORES = 8
BPC = B // N_CORES  # samples per core
N_HALF = C // 128   # channel halves (partition tiles)
EPS = 1e-5
MIN_COUNT = 100.0

IO_DT = "float16"   # HBM dtype for x / out
TAB_DT = "float32"  # dtype of the scale/shift table

_PROGRAM_CACHE = {}
LAST_RESULTS = None  # BassKernelResults of the most recent run (for profiling)


def _np_dt(name):
    if name == "bfloat16":
        import ml_dtypes

        return np.dtype(ml_dtypes.bfloat16)
    return np.dtype(name)


def _build_program(iters=1, bufs=4, dyn_loop=None, in_place=False,
                   fuse_halves=1, split=1, obufs=None, store_swdge=False,
                   io_dt=None, tab_dt=None):
    """Build + compile the single-core SPMD Bass program (cached).

    iters > 1 repeats the identical sweep back-to-back inside one NEFF;
    dyn_loop=N wraps the sweep in a hardware For loop of N trips. Both are
    used only by the benchmark harness to measure per-sweep cost.
    in_place applies the affine into the input tile (one pool, more bufs).
    fuse_halves=G >= 1 loads/stores G whole samples (both channel halves)
    per DMA. split > 1 cuts each plane DMA into `split` free-dim chunks.
    io_dt/tab_dt override the HBM x/out dtype and table dtype.
    """
    fuse_halves = int(fuse_halves)
    obufs = bufs if obufs is None else obufs
    io_dt = IO_DT if io_dt is None else io_dt
    tab_dt = TAB_DT if tab_dt is None else tab_dt
    key = (iters, bufs, dyn_loop, in_place, fuse_halves, split, obufs,
           store_swdge, io_dt, tab_dt)
    if key in _PROGRAM_CACHE:
        return _PROGRAM_CACHE[key]

    import concourse.tile as tile
    from concourse import bacc, mybir

    fio = getattr(mybir.dt, io_dt)
    ftab = getattr(mybir.dt, tab_dt)
    nc = bacc.Bacc(
        "TRN2", target_bir_lowering=False, debug=False, num_devices=N_CORES
    )
    x_ap = nc.dram_tensor("x", [BPC, C, HW], fio, kind="ExternalInput").ap()
    tab_ap = nc.dram_tensor(
        "tables", [128, BPC * N_HALF * 2], ftab, kind="ExternalInput"
    ).ap()
    out_ap = nc.dram_tensor("out", [BPC, C, HW], fio, kind="ExternalOutput").ap()

    with tile.TileContext(nc) as tc:
        with ExitStack() as ctx:
            tabp = ctx.enter_context(tc.tile_pool(name="tab", bufs=1))
            xp = ctx.enter_context(tc.tile_pool(name="xs", bufs=bufs))
            outp = ctx.enter_context(tc.tile_pool(name="os", bufs=obufs))
            st_eng = nc.gpsimd if store_swdge else nc.sync

            tab = tabp.tile([128, BPC * N_HALF * 2], ftab)
            nc.sync.dma_start(tab[:], tab_ap[:])

            def sweep():
                if fuse_halves:
                    G = fuse_halves  # samples per tile
                    for b0 in range(0, BPC, G):
                        t = xp.tile([128, G * N_HALF, HW], fio)
                        src = x_ap[b0 : b0 + G].rearrange(
                            "g (h p) f -> p (g h) f", h=N_HALF
                        )
                        nc.sync.dma_start(t[:], src)
                        o = t if in_place else outp.tile([128, G * N_HALF, HW], fio)
                        for j in range(G * N_HALF):
                            r = N_HALF * b0 + j
                            nc.vector.tensor_scalar(
                                o[:, j, :],
                                t[:, j, :],
                                tab[:, 2 * r : 2 * r + 1],
                                tab[:, 2 * r + 1 : 2 * r + 2],
                                mybir.AluOpType.mult,
                                mybir.AluOpType.add,
                            )
                        dst = out_ap[b0 : b0 + G].rearrange(
                            "g (h p) f -> p (g h) f", h=N_HALF
                        )
                        st_eng.dma_start(dst, o[:])
                    return
                fw = HW // split
                for b in range(BPC):
                    for h in range(N_HALF):
                        for s in range(split):
                            r = N_HALF * b + h
                            t = xp.tile([128, fw], fio)
                            nc.sync.dma_start(
                                t[:],
                                x_ap[b, 128 * h : 128 * (h + 1),
                                     s * fw : (s + 1) * fw],
                            )
                            o = t if in_place else outp.tile([128, fw], fio)
                            nc.vector.tensor_scalar(
                                o[:],
                                t[:],
                                tab[:, 2 * r : 2 * r + 1],
                                tab[:, 2 * r + 1 : 2 * r + 2],
                                mybir.AluOpType.mult,
                                mybir.AluOpType.add,
                            )
                            nc.sync.dma_start(
                                out_ap[b, 128 * h : 128 * (h + 1),
                                       s * fw : (s + 1) * fw],
                                o[:],
                            )

            if dyn_loop is not None:
                with tc.For_i(0, dyn_loop, 1):
                    for _ in range(iters):
                        sweep()
            else:
                for _ in range(iters):
                    sweep()

    nc.compile()
    _PROGRAM_CACHE[key] = nc
    return nc


def _scale_shift(labels, weight, bias, global_mean, global_var,
                 class_mean, class_var, class_counts, alpha):
    """Per-sample affine tables [B, C], mirroring the reference's f32 branch
    selection exactly; the weight/sqrt fold is done in f64 for accuracy."""
    labels = np.asarray(labels).astype(np.int64).reshape(-1)
    a = np.float32(np.asarray(alpha).reshape(()))
    one_m_a = np.float32(1.0) - a

    use_class = (float(a) > 0.0) & (
        np.asarray(class_counts, np.float32)[labels] >= np.float32(MIN_COUNT)
    )  # [B]
    gm = np.asarray(global_mean, np.float32)
    gv = np.asarray(global_var, np.float32)
    blend_mean = one_m_a * gm[None, :] + a * np.asarray(class_mean, np.float32)[labels]
    blend_var = np.clip(
        one_m_a * gv[None, :] + a * np.asarray(class_var, np.float32)[labels],
        np.float32(EPS),
        None,
    )
    mean = np.where(use_class[:, None], blend_mean, gm[None, :])  # [B, C] f32
    var = np.where(use_class[:, None], blend_var, gv[None, :])

    scale64 = np.asarray(weight, np.float64)[None, :] / np.sqrt(
        var.astype(np.float64) + np.float64(EPS)
    )
    shift64 = np.asarray(bias, np.float64)[None, :] - mean.astype(np.float64) * scale64
    return scale64.astype(np.float32), shift64.astype(np.float32)


def make_in_maps(inputs, io_dt=None, tab_dt=None):
    """Per-core input dicts for the SPMD program from the FULL input dict."""
    io_np = _np_dt(IO_DT if io_dt is None else io_dt)
    tab_np = _np_dt(TAB_DT if tab_dt is None else tab_dt)
    x = np.asarray(inputs["x"], np.float32)
    scale, shift = _scale_shift(**{k: inputs[k] for k in (
        "labels", "weight", "bias", "global_mean", "global_var",
        "class_mean", "class_var", "class_counts", "alpha")})

    x_io = np.ascontiguousarray(x.reshape(B, C, HW)).astype(io_np)
    in_maps = []
    for c in range(N_CORES):
        xs = x_io[c * BPC : (c + 1) * BPC]
        sc = scale[c * BPC : (c + 1) * BPC].reshape(BPC, N_HALF, 128)
        sh = shift[c * BPC : (c + 1) * BPC].reshape(BPC, N_HALF, 128)
        st = np.stack([sc, sh], axis=-1)  # [b, h, p, 2]
        tab = np.ascontiguousarray(
            st.transpose(2, 0, 1, 3).reshape(128, BPC * N_HALF * 2)
        ).astype(tab_np)  # col = 4b + 2h + k
        in_maps.append({"x": np.ascontiguousarray(xs), "tables": tab})
    return in_maps


def assemble_output(per_core_out):
    """[N_CORES, BPC, C, HW] (any float dtype) -> full [B, C, H, W] f32."""
    out = np.asarray(per_core_out).astype(np.float32)
    return out.reshape(B, C, H, W)


def kernel(x, labels, weight, bias, global_mean, global_var,
           class_mean, class_var, class_counts, alpha):
    global LAST_RESULTS
    from concourse.bass_utils import run_bass_kernel_spmd

    in_maps = make_in_maps(dict(
        x=x, labels=labels, weight=weight, bias=bias,
        global_mean=global_mean, global_var=global_var,
        class_mean=class_mean, class_var=class_var,
        class_counts=class_counts, alpha=alpha,
    ))
    nc = _build_program()

    res = run_bass_kernel_spmd(nc, in_maps, list(range(N_CORES)))
    LAST_RESULTS = res

    return assemble_output(
        np.stack([res.results[c]["out"] for c in range(N_CORES)])
    )


# revision 14
# speedup vs baseline: 1.0405x; 1.0405x over previous
"""Eval-mode ClassConditionalBatchNorm2d on 8 Trainium2 NeuronCores.

Math: for each sample b with label l:
    use_class = (alpha > 0) & (class_counts[l] >= 100)
    mean/var  = blend of (global, class[l]) stats if use_class else global
    out       = (x - mean) / sqrt(var + eps) * weight + bias

This folds to a per-(sample, channel) affine:  out = x * scale + shift with
    scale[b,c] = weight[c] / sqrt(var[b,c] + eps)
    shift[b,c] = bias[c] - mean[b,c] * scale[b,c]

The [B=64, C=256] scale/shift tables are tiny and computed on host; the
device kernel streams x through SBUF applying one fused DVE tensor_scalar
(mult+add, per-partition scalars) per channel-half — purely memory bound.

Traffic optimization: the accuracy budget (rel err < 2e-2 vs the f32
reference) is far looser than f32, so x crosses HBM as float16 (host casts
x -> f16, device reads f16, computes via DVE, writes f16 out, host upcasts).
f16 rounding of x and out each contribute <= 2^-11 relative error (~6e-4
scale-relative total) while halving HBM bytes: 12.85 MB in + 12.85 MB out
per core instead of 25.7 + 25.7. The scale/shift table must stay f32
(tensor_scalar requires f32 scalar operands).

Sharding: pure data parallel over batch. Each of the 8 cores gets 8 samples
(x shard [8, 256, 56*56]) plus its own [128, 32] scale/shift table arranged
so that column 4*b + 2*h + {0,1} holds (scale, shift) for sample b, channel
half h, with channels on partitions. Tiles cover one whole sample
([128 partitions, 2 halves, 3136 spatial] = 1.6 MB f16) so each load/store
is a single large DMA that fans across the SDMA ports; 4 input + 3 output
buffers pipeline load/compute/store.

Measured (differenced hardware-For-loop wall-clock, drift-robust interleaved
sampling): ~78.5 us/sweep vs the ~71.8 us f16 roofline at 358 GB/s — i.e.
~325 GB/s combined read+write, which matches the achievable mixed R/W HBM
rate (read-only measures ~358 GB/s; the f32 version of this same pipeline
measures ~156 us = ~330 GB/s mixed). Scheduling variants explored and
rejected under the robust bench: smaller tiles (32 DMAs: +10 us), bigger
fused tiles (8 DMAs of 3.2 MB: +2 us), stores on the second HWDGE ring
(qActDynamicHW: +1 us), in-place compute with deep pools (unstable timing,
no gain). The pipeline is at the memory floor; only fewer bytes would go
faster, and fp8 fails the 2e-2 max-error gate (2^-4 mantissa step = 6.25%).
"""

import numpy as np
from contextlib import ExitStack

B, C, H, W = 64, 256, 56, 56
HW = H * W
N_CORES = 8
BPC = B // N_CORES  # samples per core
N_HALF = C // 128   # channel halves (partition tiles)
EPS = 1e-5
MIN_COUNT = 100.0

IO_DT = "float16"   # HBM dtype for x / out
TAB_DT = "float32"  # dtype of the scale/shift table

_PROGRAM_CACHE = {}
LAST_RESULTS = None  # BassKernelResults of the most recent run (for profiling)


def _np_dt(name):
    if name == "bfloat16":
        import ml_dtypes

        return np.dtype(ml_dtypes.bfloat16)
    return np.dtype(name)


def _build_program(iters=1, bufs=4, dyn_loop=None, in_place=False,
                   fuse_halves=1, split=1, obufs=None, store_swdge=False,
                   io_dt=None, tab_dt=None, store_eng="sync", load_eng="sync",
                   mode="normal"):
    """Build + compile the single-core SPMD Bass program (cached).

    iters > 1 repeats the identical sweep back-to-back inside one NEFF;
    dyn_loop=N wraps the sweep in a hardware For loop of N trips. Both are
    used only by the benchmark harness to measure per-sweep cost.
    in_place applies the affine into the input tile (one pool, more bufs).
    fuse_halves=G >= 1 loads/stores G whole samples (both channel halves)
    per DMA. split > 1 cuts each plane DMA into `split` free-dim chunks.
    io_dt/tab_dt override the HBM x/out dtype and table dtype.
    """
    fuse_halves = int(fuse_halves)
    obufs = bufs if obufs is None else obufs
    io_dt = IO_DT if io_dt is None else io_dt
    tab_dt = TAB_DT if tab_dt is None else tab_dt
    key = (iters, bufs, dyn_loop, in_place, fuse_halves, split, obufs,
           store_swdge, io_dt, tab_dt, store_eng, load_eng, mode)
    if key in _PROGRAM_CACHE:
        return _PROGRAM_CACHE[key]

    import concourse.tile as tile
    from concourse import bacc, mybir

    fio = getattr(mybir.dt, io_dt)
    ftab = getattr(mybir.dt, tab_dt)
    nc = bacc.Bacc(
        "TRN2", target_bir_lowering=False, debug=False, num_devices=N_CORES
    )
    x_ap = nc.dram_tensor("x", [BPC, C, HW], fio, kind="ExternalInput").ap()
    tab_ap = nc.dram_tensor(
        "tables", [128, BPC * N_HALF * 2], ftab, kind="ExternalInput"
    ).ap()
    out_ap = nc.dram_tensor("out", [BPC, C, HW], fio, kind="ExternalOutput").ap()

    with tile.TileContext(nc) as tc:
        with ExitStack() as ctx:
            tabp = ctx.enter_context(tc.tile_pool(name="tab", bufs=1))
            xp = ctx.enter_context(tc.tile_pool(name="xs", bufs=bufs))
            outp = ctx.enter_context(tc.tile_pool(name="os", bufs=obufs))
            st_eng = nc.gpsimd if store_swdge else getattr(nc, store_eng)
            ld_eng = getattr(nc, load_eng)

            # Table DMA on the Act HWDGE ring: overlaps the first x load
            # (which streams on the SP ring) in the single-shot execution.
            tab = tabp.tile([128, BPC * N_HALF * 2], ftab)
            nc.scalar.dma_start(tab[:], tab_ap[:])

            # micro-bench modes: one shared tile, stream one direction only
            if mode != "normal":
                microp = ctx.enter_context(tc.tile_pool(name="micro", bufs=1))
                micro = microp.tile([128, N_HALF, HW], fio)

            def sweep():
                if mode == "load_only":
                    for b in range(BPC):
                        t = xp.tile([128, N_HALF, HW], fio)
                        ld_eng.dma_start(
                            t[:], x_ap[b].rearrange("(h p) f -> p h f", h=N_HALF)
                        )
                    return
                if mode == "store_only":
                    for b in range(BPC):
                        st_eng.dma_start(
                            out_ap[b].rearrange("(h p) f -> p h f", h=N_HALF),
                            micro[:],
                        )
                    return
                if mode == "copy":
                    for b in range(BPC):
                        t = xp.tile([128, N_HALF, HW], fio)
                        ld_eng.dma_start(
                            t[:], x_ap[b].rearrange("(h p) f -> p h f", h=N_HALF)
                        )
                        st_eng.dma_start(
                            out_ap[b].rearrange("(h p) f -> p h f", h=N_HALF),
                            t[:],
                        )
                    return
                if fuse_halves:
                    G = fuse_halves  # samples per tile
                    for b0 in range(0, BPC, G):
                        t = xp.tile([128, G * N_HALF, HW], fio)
                        src = x_ap[b0 : b0 + G].rearrange(
                            "g (h p) f -> p (g h) f", h=N_HALF
                        )
                        ld_eng.dma_start(t[:], src)
                        o = t if in_place else outp.tile([128, G * N_HALF, HW], fio)
                        for j in range(G * N_HALF):
                            r = N_HALF * b0 + j
                            nc.vector.tensor_scalar(
                                o[:, j, :],
                                t[:, j, :],
                                tab[:, 2 * r : 2 * r + 1],
                                tab[:, 2 * r + 1 : 2 * r + 2],
                                mybir.AluOpType.mult,
                                mybir.AluOpType.add,
                            )
                        dst = out_ap[b0 : b0 + G].rearrange(
                            "g (h p) f -> p (g h) f", h=N_HALF
                        )
                        st_eng.dma_start(dst, o[:])
                    return
                fw = HW // split
                for b in range(BPC):
                    for h in range(N_HALF):
                        for s in range(split):
                            r = N_HALF * b + h
                            t = xp.tile([128, fw], fio)
                            ld_eng.dma_start(
                                t[:],
                                x_ap[b, 128 * h : 128 * (h + 1),
                                     s * fw : (s + 1) * fw],
                            )
                            o = t if in_place else outp.tile([128, fw], fio)
                            nc.vector.tensor_scalar(
                                o[:],
                                t[:],
                                tab[:, 2 * r : 2 * r + 1],
                                tab[:, 2 * r + 1 : 2 * r + 2],
                                mybir.AluOpType.mult,
                                mybir.AluOpType.add,
                            )
                            st_eng.dma_start(
                                out_ap[b, 128 * h : 128 * (h + 1),
                                       s * fw : (s + 1) * fw],
                                o[:],
                            )

            if dyn_loop is not None:
                with tc.For_i(0, dyn_loop, 1):
                    for _ in range(iters):
                        sweep()
            else:
                for _ in range(iters):
                    sweep()

    nc.compile()
    _PROGRAM_CACHE[key] = nc
    return nc


def _scale_shift(labels, weight, bias, global_mean, global_var,
                 class_mean, class_var, class_counts, alpha):
    """Per-sample affine tables [B, C], mirroring the reference's f32 branch
    selection exactly; the weight/sqrt fold is done in f64 for accuracy."""
    labels = np.asarray(labels).astype(np.int64).reshape(-1)
    a = np.float32(np.asarray(alpha).reshape(()))
    one_m_a = np.float32(1.0) - a

    use_class = (float(a) > 0.0) & (
        np.asarray(class_counts, np.float32)[labels] >= np.float32(MIN_COUNT)
    )  # [B]
    gm = np.asarray(global_mean, np.float32)
    gv = np.asarray(global_var, np.float32)
    blend_mean = one_m_a * gm[None, :] + a * np.asarray(class_mean, np.float32)[labels]
    blend_var = np.clip(
        one_m_a * gv[None, :] + a * np.asarray(class_var, np.float32)[labels],
        np.float32(EPS),
        None,
    )
    mean = np.where(use_class[:, None], blend_mean, gm[None, :])  # [B, C] f32
    var = np.where(use_class[:, None], blend_var, gv[None, :])

    scale64 = np.asarray(weight, np.float64)[None, :] / np.sqrt(
        var.astype(np.float64) + np.float64(EPS)
    )
    shift64 = np.asarray(bias, np.float64)[None, :] - mean.astype(np.float64) * scale64
    return scale64.astype(np.float32), shift64.astype(np.float32)


def make_in_maps(inputs, io_dt=None, tab_dt=None):
    """Per-core input dicts for the SPMD program from the FULL input dict."""
    io_np = _np_dt(IO_DT if io_dt is None else io_dt)
    tab_np = _np_dt(TAB_DT if tab_dt is None else tab_dt)
    x = np.asarray(inputs["x"], np.float32)
    scale, shift = _scale_shift(**{k: inputs[k] for k in (
        "labels", "weight", "bias", "global_mean", "global_var",
        "class_mean", "class_var", "class_counts", "alpha")})

    x_io = np.ascontiguousarray(x.reshape(B, C, HW)).astype(io_np)
    in_maps = []
    for c in range(N_CORES):
        xs = x_io[c * BPC : (c + 1) * BPC]
        sc = scale[c * BPC : (c + 1) * BPC].reshape(BPC, N_HALF, 128)
        sh = shift[c * BPC : (c + 1) * BPC].reshape(BPC, N_HALF, 128)
        st = np.stack([sc, sh], axis=-1)  # [b, h, p, 2]
        tab = np.ascontiguousarray(
            st.transpose(2, 0, 1, 3).reshape(128, BPC * N_HALF * 2)
        ).astype(tab_np)  # col = 4b + 2h + k
        in_maps.append({"x": np.ascontiguousarray(xs), "tables": tab})
    return in_maps


def assemble_output(per_core_out):
    """[N_CORES, BPC, C, HW] (any float dtype) -> full [B, C, H, W] f32."""
    out = np.asarray(per_core_out).astype(np.float32)
    return out.reshape(B, C, H, W)


def kernel(x, labels, weight, bias, global_mean, global_var,
           class_mean, class_var, class_counts, alpha):
    global LAST_RESULTS
    from concourse.bass_utils import run_bass_kernel_spmd

    in_maps = make_in_maps(dict(
        x=x, labels=labels, weight=weight, bias=bias,
        global_mean=global_mean, global_var=global_var,
        class_mean=class_mean, class_var=class_var,
        class_counts=class_counts, alpha=alpha,
    ))
    nc = _build_program()

    res = run_bass_kernel_spmd(nc, in_maps, list(range(N_CORES)))
    LAST_RESULTS = res

    return assemble_output(
        np.stack([res.results[c]["out"] for c in range(N_CORES)])
    )


# revision 17
# speedup vs baseline: 1.0969x; 1.0541x over previous
"""Eval-mode ClassConditionalBatchNorm2d on 8 Trainium2 NeuronCores.

Math: for each sample b with label l:
    use_class = (alpha > 0) & (class_counts[l] >= 100)
    mean/var  = blend of (global, class[l]) stats if use_class else global
    out       = (x - mean) / sqrt(var + eps) * weight + bias

This folds to a per-(sample, channel) affine:  out = x * scale + shift with
    scale[b,c] = weight[c] / sqrt(var[b,c] + eps)
    shift[b,c] = bias[c] - mean[b,c] * scale[b,c]

The [B=64, C=256] scale/shift tables are tiny and computed on host; the
device kernel streams x through SBUF applying one fused DVE tensor_scalar
(mult+add, per-partition scalars) per channel-half — purely memory bound.

Traffic optimization: the accuracy budget (rel err < 2e-2 vs the f32
reference) is far looser than f32, so x crosses HBM as float16 (host casts
x -> f16, device reads f16, computes via DVE, writes f16 out, host upcasts).
f16 rounding of x and out each contribute <= 2^-11 relative error (~6e-4
scale-relative total) while halving HBM bytes: 12.85 MB in + 12.85 MB out
per core instead of 25.7 + 25.7. The scale/shift table must stay f32
(tensor_scalar requires f32 scalar operands).

Sharding: pure data parallel over batch. Each of the 8 cores gets 8 samples
(x shard [8, 256, 56*56]) plus its own [128, 32] scale/shift table arranged
so that column 4*b + 2*h + {0,1} holds (scale, shift) for sample b, channel
half h, with channels on partitions. Tiles cover one whole sample
([128 partitions, 2 halves, 3136 spatial] = 1.6 MB f16) so each load/store
is a single large DMA that fans across the SDMA ports; 4 input + 3 output
buffers pipeline load/compute/store.

Schedule: PHASE-SEPARATED sweep ("phased" mode). Fine-grained read/write
interleaving caps HBM at ~325 GB/s combined (measured; read-only hits
~358 GB/s), so the sweep enqueues all 8 sample loads back-to-back on the SP
HWDGE ring (pure-read phase at full read rate), DVE computes in place
trailing the loads, then all 8 stores drain as a pure-write phase behind
them in the same FIFO. Measured ~76.8 us/sweep vs ~80 us for the classic
interleaved load/compute/store pipeline and vs the 71.8 us f16 roofline
at 358 GB/s. Rejected under the drift-robust bench: smaller tiles (32 DMAs:
+10 us), bigger fused tiles, stores on the second HWDGE ring, deep-pool
in-place interleaved variants. Only fewer bytes would go materially faster,
and fp8 fails the 2e-2 max-error gate (2^-4 mantissa step = 6.25%).
"""

import numpy as np
from contextlib import ExitStack

B, C, H, W = 64, 256, 56, 56
HW = H * W
N_CORES = 8
BPC = B // N_CORES  # samples per core
N_HALF = C // 128   # channel halves (partition tiles)
EPS = 1e-5
MIN_COUNT = 100.0

IO_DT = "float16"   # HBM dtype for x / out
TAB_DT = "float32"  # dtype of the scale/shift table

_PROGRAM_CACHE = {}
LAST_RESULTS = None  # BassKernelResults of the most recent run (for profiling)


def _np_dt(name):
    if name == "bfloat16":
        import ml_dtypes

        return np.dtype(ml_dtypes.bfloat16)
    return np.dtype(name)


def _build_program(iters=1, bufs=4, dyn_loop=None, in_place=False,
                   fuse_halves=1, split=1, obufs=None, store_swdge=False,
                   io_dt=None, tab_dt=None, store_eng="sync", load_eng="sync",
                   mode="normal"):
    """Build + compile the single-core SPMD Bass program (cached).

    iters > 1 repeats the identical sweep back-to-back inside one NEFF;
    dyn_loop=N wraps the sweep in a hardware For loop of N trips. Both are
    used only by the benchmark harness to measure per-sweep cost.
    in_place applies the affine into the input tile (one pool, more bufs).
    fuse_halves=G >= 1 loads/stores G whole samples (both channel halves)
    per DMA. split > 1 cuts each plane DMA into `split` free-dim chunks.
    io_dt/tab_dt override the HBM x/out dtype and table dtype.
    """
    fuse_halves = int(fuse_halves)
    obufs = bufs if obufs is None else obufs
    io_dt = IO_DT if io_dt is None else io_dt
    tab_dt = TAB_DT if tab_dt is None else tab_dt
    key = (iters, bufs, dyn_loop, in_place, fuse_halves, split, obufs,
           store_swdge, io_dt, tab_dt, store_eng, load_eng, mode)
    if key in _PROGRAM_CACHE:
        return _PROGRAM_CACHE[key]

    import concourse.tile as tile
    from concourse import bacc, mybir

    fio = getattr(mybir.dt, io_dt)
    ftab = getattr(mybir.dt, tab_dt)
    nc = bacc.Bacc(
        "TRN2", target_bir_lowering=False, debug=False, num_devices=N_CORES
    )
    x_ap = nc.dram_tensor("x", [BPC, C, HW], fio, kind="ExternalInput").ap()
    tab_ap = nc.dram_tensor(
        "tables", [128, BPC * N_HALF * 2], ftab, kind="ExternalInput"
    ).ap()
    out_ap = nc.dram_tensor("out", [BPC, C, HW], fio, kind="ExternalOutput").ap()

    with tile.TileContext(nc) as tc:
        with ExitStack() as ctx:
            tabp = ctx.enter_context(tc.tile_pool(name="tab", bufs=1))
            xp = ctx.enter_context(tc.tile_pool(name="xs", bufs=bufs))
            outp = ctx.enter_context(tc.tile_pool(name="os", bufs=obufs))
            st_eng = nc.gpsimd if store_swdge else getattr(nc, store_eng)
            ld_eng = getattr(nc, load_eng)

            # Table DMA on the Act HWDGE ring: overlaps the first x load
            # (which streams on the SP ring) in the single-shot execution.
            tab = tabp.tile([128, BPC * N_HALF * 2], ftab)
            nc.scalar.dma_start(tab[:], tab_ap[:])

            # micro-bench modes: one shared tile, stream one direction only
            if mode != "normal":
                microp = ctx.enter_context(tc.tile_pool(name="micro", bufs=1))
                micro = microp.tile([128, N_HALF, HW], fio)

            def sweep():
                if mode == "phased":
                    # Phase-separated sweep: all loads enqueue on the SP ring
                    # first (pure-read phase at full HBM read rate), DVE
                    # computes in place trailing the loads, then all stores
                    # drain as a pure-write phase. Avoids the fine-grained
                    # R/W interleave that caps mixed traffic at ~325 GB/s.
                    tiles = []
                    for b in range(BPC):
                        t = xp.tile([128, N_HALF, HW], fio)
                        ld_eng.dma_start(
                            t[:], x_ap[b].rearrange("(h p) f -> p h f", h=N_HALF)
                        )
                        for h in range(N_HALF):
                            r = N_HALF * b + h
                            nc.vector.tensor_scalar(
                                t[:, h, :],
                                t[:, h, :],
                                tab[:, 2 * r : 2 * r + 1],
                                tab[:, 2 * r + 1 : 2 * r + 2],
                                mybir.AluOpType.mult,
                                mybir.AluOpType.add,
                            )
                        tiles.append(t)
                    for b in range(BPC):
                        st_eng.dma_start(
                            out_ap[b].rearrange("(h p) f -> p h f", h=N_HALF),
                            tiles[b][:],
                        )
                    return
                if mode == "load_only":
                    for b in range(BPC):
                        t = xp.tile([128, N_HALF, HW], fio)
                        ld_eng.dma_start(
                            t[:], x_ap[b].rearrange("(h p) f -> p h f", h=N_HALF)
                        )
                    return
                if mode == "store_only":
                    for b in range(BPC):
                        st_eng.dma_start(
                            out_ap[b].rearrange("(h p) f -> p h f", h=N_HALF),
                            micro[:],
                        )
                    return
                if mode == "copy":
                    for b in range(BPC):
                        t = xp.tile([128, N_HALF, HW], fio)
                        ld_eng.dma_start(
                            t[:], x_ap[b].rearrange("(h p) f -> p h f", h=N_HALF)
                        )
                        st_eng.dma_start(
                            out_ap[b].rearrange("(h p) f -> p h f", h=N_HALF),
                            t[:],
                        )
                    return
                if fuse_halves:
                    G = fuse_halves  # samples per tile
                    for b0 in range(0, BPC, G):
                        t = xp.tile([128, G * N_HALF, HW], fio)
                        src = x_ap[b0 : b0 + G].rearrange(
                            "g (h p) f -> p (g h) f", h=N_HALF
                        )
                        ld_eng.dma_start(t[:], src)
                        o = t if in_place else outp.tile([128, G * N_HALF, HW], fio)
                        for j in range(G * N_HALF):
                            r = N_HALF * b0 + j
                            nc.vector.tensor_scalar(
                                o[:, j, :],
                                t[:, j, :],
                                tab[:, 2 * r : 2 * r + 1],
                                tab[:, 2 * r + 1 : 2 * r + 2],
                                mybir.AluOpType.mult,
                                mybir.AluOpType.add,
                            )
                        dst = out_ap[b0 : b0 + G].rearrange(
                            "g (h p) f -> p (g h) f", h=N_HALF
                        )
                        st_eng.dma_start(dst, o[:])
                    return
                fw = HW // split
                for b in range(BPC):
                    for h in range(N_HALF):
                        for s in range(split):
                            r = N_HALF * b + h
                            t = xp.tile([128, fw], fio)
                            ld_eng.dma_start(
                                t[:],
                                x_ap[b, 128 * h : 128 * (h + 1),
                                     s * fw : (s + 1) * fw],
                            )
                            o = t if in_place else outp.tile([128, fw], fio)
                            nc.vector.tensor_scalar(
                                o[:],
                                t[:],
                                tab[:, 2 * r : 2 * r + 1],
                                tab[:, 2 * r + 1 : 2 * r + 2],
                                mybir.AluOpType.mult,
                                mybir.AluOpType.add,
                            )
                            st_eng.dma_start(
                                out_ap[b, 128 * h : 128 * (h + 1),
                                       s * fw : (s + 1) * fw],
                                o[:],
                            )

            if dyn_loop is not None:
                with tc.For_i(0, dyn_loop, 1):
                    for _ in range(iters):
                        sweep()
            else:
                for _ in range(iters):
                    sweep()

    nc.compile()
    _PROGRAM_CACHE[key] = nc
    return nc


def _scale_shift(labels, weight, bias, global_mean, global_var,
                 class_mean, class_var, class_counts, alpha):
    """Per-sample affine tables [B, C], mirroring the reference's f32 branch
    selection exactly; the weight/sqrt fold is done in f64 for accuracy."""
    labels = np.asarray(labels).astype(np.int64).reshape(-1)
    a = np.float32(np.asarray(alpha).reshape(()))
    one_m_a = np.float32(1.0) - a

    use_class = (float(a) > 0.0) & (
        np.asarray(class_counts, np.float32)[labels] >= np.float32(MIN_COUNT)
    )  # [B]
    gm = np.asarray(global_mean, np.float32)
    gv = np.asarray(global_var, np.float32)
    blend_mean = one_m_a * gm[None, :] + a * np.asarray(class_mean, np.float32)[labels]
    blend_var = np.clip(
        one_m_a * gv[None, :] + a * np.asarray(class_var, np.float32)[labels],
        np.float32(EPS),
        None,
    )
    mean = np.where(use_class[:, None], blend_mean, gm[None, :])  # [B, C] f32
    var = np.where(use_class[:, None], blend_var, gv[None, :])

    scale64 = np.asarray(weight, np.float64)[None, :] / np.sqrt(
        var.astype(np.float64) + np.float64(EPS)
    )
    shift64 = np.asarray(bias, np.float64)[None, :] - mean.astype(np.float64) * scale64
    return scale64.astype(np.float32), shift64.astype(np.float32)


def make_in_maps(inputs, io_dt=None, tab_dt=None):
    """Per-core input dicts for the SPMD program from the FULL input dict."""
    io_np = _np_dt(IO_DT if io_dt is None else io_dt)
    tab_np = _np_dt(TAB_DT if tab_dt is None else tab_dt)
    x = np.asarray(inputs["x"], np.float32)
    scale, shift = _scale_shift(**{k: inputs[k] for k in (
        "labels", "weight", "bias", "global_mean", "global_var",
        "class_mean", "class_var", "class_counts", "alpha")})

    x_io = np.ascontiguousarray(x.reshape(B, C, HW)).astype(io_np)
    in_maps = []
    for c in range(N_CORES):
        xs = x_io[c * BPC : (c + 1) * BPC]
        sc = scale[c * BPC : (c + 1) * BPC].reshape(BPC, N_HALF, 128)
        sh = shift[c * BPC : (c + 1) * BPC].reshape(BPC, N_HALF, 128)
        st = np.stack([sc, sh], axis=-1)  # [b, h, p, 2]
        tab = np.ascontiguousarray(
            st.transpose(2, 0, 1, 3).reshape(128, BPC * N_HALF * 2)
        ).astype(tab_np)  # col = 4b + 2h + k
        in_maps.append({"x": np.ascontiguousarray(xs), "tables": tab})
    return in_maps


def assemble_output(per_core_out):
    """[N_CORES, BPC, C, HW] (any float dtype) -> full [B, C, H, W] f32."""
    out = np.asarray(per_core_out).astype(np.float32)
    return out.reshape(B, C, H, W)


def kernel(x, labels, weight, bias, global_mean, global_var,
           class_mean, class_var, class_counts, alpha):
    global LAST_RESULTS
    from concourse.bass_utils import run_bass_kernel_spmd

    in_maps = make_in_maps(dict(
        x=x, labels=labels, weight=weight, bias=bias,
        global_mean=global_mean, global_var=global_var,
        class_mean=class_mean, class_var=class_var,
        class_counts=class_counts, alpha=alpha,
    ))
    nc = _build_program(bufs=8, mode="phased")

    res = run_bass_kernel_spmd(nc, in_maps, list(range(N_CORES)))
    LAST_RESULTS = res

    return assemble_output(
        np.stack([res.results[c]["out"] for c in range(N_CORES)])
    )
